# revision 5
# baseline (speedup 1.0000x reference)
"""Trainium2 Bass kernel for nn_BernoulliDecompAttModel (decomposable attention NLI model).

Contract: kernel(**inputs) takes the FULL unsharded inputs (as produced by
setup_inputs()) and returns the FULL [64, 3] float32 output. Internally the
batch (64) is sharded 8-ways across 8 NeuronCores (pure data parallel, all
weights replicated); each core runs an identical Bass/Tile program on its 8
batch items.

v2 (416us -> ?): key-major attention via score symmetry. Q.Q^T is symmetric
and z^T is the operand-swapped matmul, so every attention matrix is computed
directly with KEYS on partitions:
  - key masks become per-partition activation biases on the exp (no more
    ones[1,128] mask-inject matmuls),
  - the relative-distance bias enters the score PSUM as one identity matmul
    of a host-precomputed transposed-Toeplitz biasT tile,
  - softmax denominators = partition sums via ones_col[128,1] matmuls
    accumulated across both key chunks,
  - normalization happens in the natural layout: attT = ET * bcast(1/sum)
    with the reciprocal row broadcast across partitions on the idle GPSIMD
    engine (partition_broadcast), multiplied on DVE straight to fp8.
This deletes all is_transpose matmuls (attT/p2hT/h2pT/zT), all mask matmuls,
the f32r PSUM trick, and the madd/idxrow path. Weights are host-packed into
their exact SBUF layouts (single dense DMA each, biases packed in one
tensor, biasT Toeplitz built on host). A burst of identity warm-up matmuls
trips the HAM activity window during the initial weight/token DMA so real
matmuls start at 2.4 GHz. Scalar/vector work is split ~evenly.

fp8-e4m3 operands + DoubleRow / DoubleRowSwInterleave on all numerically
tolerant matmuls (self/inter MLPs, compare-L1 + Y, ctx, scores); fp16 on
cmp2 and the aggregate head. PSUM accumulation f32; softmax skips
max-subtraction (scores bounded, exp cannot overflow).

Layout conventions unchanged from v1: activations feature-major
[128 partitions = feature chunk, K/128 chunks, tokens on free dim]; prem and
hypo concatenated on the token axis for shared-weight MLPs (N=512).
"""

import numpy as np
import os

B, L, V, E, D, OUT = 64, 256, 50000, 512, 512, 3
NCORES = 8
BL = B // NCORES            # batch items per core
MAX_DIST = 11
MASK_VAL = -30000.0         # padded-key additive mask (exp() underflows to 0)
DIAG_VAL = -30000.0         # self-attention diagonal (fp16-safe; exp() -> 0)

_PROG_CACHE = {}


def _build_program(debug_taps=()):
    import concourse.bass as bass
    import concourse.bacc as bacc
    import concourse.mybir as mybir
    from concourse.tile import TileContext
    from concourse.masks import make_identity

    dt = mybir.dt
    f32, f32r, i32 = dt.float32, dt.float32r, dt.int32
    hf, f8 = dt.float16, dt.float8e4
    DR = mybir.MatmulPerfMode.DoubleRow
    DRI = mybir.MatmulPerfMode.DoubleRowSwInterleave
    AF = mybir.ActivationFunctionType
    ALU = mybir.AluOpType

    PB_MODE = os.environ.get('KPBCAST', 'gp')  # 'gp' gpsimd | 'pe' outer-product

    nc = bacc.Bacc("TRN2", target_bir_lowering=False, debug=True)

    # ---------------- DRAM I/O (all weights host-packed to SBUF layout) ----
    tok = nc.dram_tensor("tok", [2, BL, L], i32, kind="ExternalInput")
    emb = nc.dram_tensor("emb", [V, E], f8, kind="ExternalInput")
    w_s1 = nc.dram_tensor("w_s1", [128, E // 256, 4, 256], f8, kind="ExternalInput")
    w_s2 = nc.dram_tensor("w_s2", [128, D // 256, 4, 256], f8, kind="ExternalInput")
    w_a1 = nc.dram_tensor("w_a1", [128, 2 * E // 256, 4, 256], f8, kind="ExternalInput")
    w_a2 = nc.dram_tensor("w_a2", [128, D // 256, 4, 256], f8, kind="ExternalInput")
    w_c1t = nc.dram_tensor("w_c1t", [128, 2 * E // 256, 4, 256], f8, kind="ExternalInput")
    w_c1b = nc.dram_tensor("w_c1b", [128, 8, 512], f8, kind="ExternalInput")
    w_c2 = nc.dram_tensor("w_c2", [128, 4, 512], hf, kind="ExternalInput")
    w_g1 = nc.dram_tensor("w_g1", [128, 8, 512], hf, kind="ExternalInput")
    w_g2 = nc.dram_tensor("w_g2", [128, 4, 512], hf, kind="ExternalInput")
    w_o = nc.dram_tensor("w_o", [128, 4, 4], hf, kind="ExternalInput")
    biases = nc.dram_tensor("biases", [128, 8, 4], f32, kind="ExternalInput")
    bc2_d = nc.dram_tensor("bc2row", [1, D], hf, kind="ExternalInput")
    biasT_d = nc.dram_tensor("biasT", [128, 2, 256], hf, kind="ExternalInput")

    out_d = nc.dram_tensor("out", [BL, OUT], f32, kind="ExternalOutput")

    dbg = {}
    for name, shape in debug_taps:
        dbg[name] = nc.dram_tensor(name, shape, f32, kind="ExternalOutput")

    with TileContext(nc) as tc:
        const = tc.alloc_tile_pool(name="const", bufs=1)
        work = tc.alloc_tile_pool(name="work", bufs=2)
        ps = tc.alloc_tile_pool(name="ps", bufs=1, space="PSUM")

        def mm512_ps(name):
            return ps.tile([128, 512], f32, space="PSUM", tag="mm512", bufs=3, name=name)

        def attn_ps(name):
            return ps.tile([128, 256], f32, space="PSUM", tag="attn", bufs=3, name=name)

        def trT_ps(name):
            return ps.tile([128, 128], f32, space="PSUM", tag="trT", bufs=2, name=name)

        def rsum_ps(name):
            return ps.tile([1, 256], f32, space="PSUM", tag="attn", bufs=3, name=name)

        # ---------------- constants ----------------
        ident32 = const.tile([128, 128], f32, name="ident32")
        make_identity(nc, ident32[:])
        ident8 = const.tile([128, 128], f8, name="ident8")
        nc.vector.tensor_copy(ident8[:], ident32[:])
        identb = const.tile([128, 128], hf, name="identb")
        nc.vector.tensor_copy(identb[:], ident32[:])
        identr = const.tile([128, 128], f32r, name="identr")
        nc.vector.tensor_copy(identr[:], ident32[:])

        ones = const.tile([1, 128], hf, name="ones")        # K=1 broadcast lhsT
        nc.vector.memset(ones[:], 1.0)
        ones_col = const.tile([128, 1], hf, name="ones_col")  # partition-sum lhsT
        nc.vector.memset(ones_col[:], 1.0)

        # HAM warm-up: ~30 back-to-back idents trip the PE activity window
        # while token/weight DMAs stream, so item-0 matmuls run at 2.4 GHz.
        for wi in range(30):
            pw = ps.tile([128, 128], f32, space="PSUM", tag="trT", bufs=2,
                         name=f"warm{wi}")
            nc.tensor.matmul(pw[:], lhsT=ident8[:], rhs=ident8[:],
                             start=True, stop=True)

        # ---------------- weights (single dense DMA each) ----------------
        def load_const(dram, shape, dtype, name):
            t = const.tile(shape, dtype, name=name)
            nc.sync.dma_start(t[:], dram.ap())
            return t

        ws1 = load_const(w_s1, [128, 2, 4, 256], f8, "ws1")
        ws2 = load_const(w_s2, [128, 2, 4, 256], f8, "ws2")
        wa1 = load_const(w_a1, [128, 4, 4, 256], f8, "wa1")
        wa2 = load_const(w_a2, [128, 2, 4, 256], f8, "wa2")
        wc1t = load_const(w_c1t, [128, 4, 4, 256], f8, "wc1t")
        wc1b = load_const(w_c1b, [128, 8, 512], f8, "wc1b")
        wc2 = load_const(w_c2, [128, 4, 512], hf, "wc2")
        wg1 = load_const(w_g1, [128, 8, 512], hf, "wg1")
        wg2 = load_const(w_g2, [128, 4, 512], hf, "wg2")
        wo = load_const(w_o, [128, 4, 4], hf, "wo")
        bsb = load_const(biases, [128, 8, 4], f32, "bsb")
        BIDX = {n: i for i, n in enumerate(
            ["b_s1", "b_s2", "b_a1", "b_a2", "b_c1", "b_c2", "b_g1", "b_g2"])}

        def bias_ap(n, nf):
            return bsb[:, BIDX[n], nf:nf + 1]

        bc2row = load_const(bc2_d, [1, D], hf, "bc2row")
        biasT = load_const(biasT_d, [128, 2, 256], hf, "biasT")

        srows = const.tile([2 * BL, 512], f32r, name="srows")

        # ---------------- helpers ----------------
        def act_relu(engine_pick, dst_ap, src_ap, bias):
            """relu(src + bias) -> dst; alternate scalar ACT / DVE tensor_scalar."""
            if engine_pick == 0:
                nc.scalar.activation(dst_ap, src_ap, AF.Relu, bias=bias)
            else:
                nc.vector.tensor_scalar(dst_ap, src_ap, bias, 0.0,
                                        op0=ALU.add, op1=ALU.max)

        def copy_ps(engine_pick, dst_ap, src_ap):
            if engine_pick == 0:
                nc.scalar.copy(dst_ap, src_ap)
            else:
                nc.vector.tensor_copy(dst_ap, src_ap)

        def mlp_fm(src, nkc, w, bname, dst, name):
            """feature-major MLP layer: dst[:,nf,:] = relu(w.T @ src + bias).
            w in the DRI interleaved layout [128, pairs, 4, 256]."""
            npair = nkc // 2
            for nf in range(4):
                pm = mm512_ps(f"{name}_nf{nf}")
                for i in range(npair):
                    nc.tensor.matmul(pm[:], lhsT=w[:, i, nf, :],
                                     rhs=src[:, 2 * i:2 * i + 2, :],
                                     start=(i == 0), stop=(i == npair - 1),
                                     perf_mode=DRI)
                act_relu(nf % 2, dst[:, nf, :], pm[:], bias_ap(bname, nf))

        def softmax_kmajor(ET, pS_by_kc, mcol_by_kc, dstT, name):
            """k-major masked softmax + normalize.

            ET[:, kc, :] = exp(pS_kc + maskcol_kc)   (keys on partitions)
            rowsum[q] = sum_k ET[k, q] via ones_col matmuls (both chunks)
            dstT[:, kc, :] = ET[:, kc, :] * bcast(1/rowsum)  -> fp8
            """
            for kc in range(2):
                nc.scalar.activation(ET[:, kc, :], pS_by_kc[kc][:], AF.Exp,
                                     bias=mcol_by_kc[kc])
            prs = rsum_ps(f"rs_{name}")
            for kc in range(2):
                nc.tensor.matmul(prs[:], lhsT=ones_col[:], rhs=ET[:, kc, :],
                                 start=(kc == 0), stop=(kc == 1))
            rec32 = work.tile([1, 256], f32, tag="rec32", bufs=4, name=f"rec32_{name}")
            nc.vector.reciprocal(rec32[:], prs[:])
            rec16 = work.tile([1, 256], hf, tag="rec16", bufs=4, name=f"rec16_{name}")
            nc.vector.tensor_copy(rec16[:], rec32[:])
            if PB_MODE == 'gp':
                recb = work.tile([128, 256], hf, tag="recb", bufs=3, name=f"recb_{name}")
                nc.gpsimd.partition_broadcast(recb[:], rec16[:], 128)
                for kc in range(2):
                    nc.vector.tensor_tensor(dstT[:, kc, :], ET[:, kc, :], recb[:],
                                            op=ALU.mult)
            else:
                prb = attn_ps(f"rb_{name}")
                nc.tensor.matmul(prb[:], lhsT=ones[:], rhs=rec16[:],
                                 start=True, stop=True)
                for kc in range(2):
                    nc.vector.tensor_tensor(dstT[:, kc, :], ET[:, kc, :], prb[:],
                                            op=ALU.mult)

        # ---------------- per-item pipeline ----------------
        nitems = int(os.environ.get('KITEMS', BL))
        STAGE = int(os.environ.get('KSTAGE', 99))
        for b in range(nitems):
            # ---- indices, masks ----
            it = work.tile([128, 2, 2], i32, tag="it", bufs=3, name=f"it{b}")
            for s in range(2):
                nc.sync.dma_start(it[:, s, :], bass.AP(tok, b * L + s * BL * L, [[1, 128], [128, 2]]))
            maskf = work.tile([128, 2, 2, 1], f8, tag="maskf", bufs=3, name=f"maskf{b}")
            nc.vector.tensor_scalar(maskf[:], it[:], 0, None, op0=ALU.not_equal)
            # additive key-mask column: -30000 where token == PAD
            mcol = work.tile([128, 2, 2], f32, tag="mcol", bufs=3, name=f"mcol{b}")
            nc.vector.tensor_scalar(mcol[:], it[:], 0, MASK_VAL,
                                    op0=ALU.is_equal, op1=ALU.mult)

            # ---- embedding gather (token-major, fp8 direct) ----
            xembs = [work.tile([128, 2, E], f8, tag=f"xembs{s}", bufs=3, name=f"xembs{b}_{s}")
                     for s in range(2)]
            for s in range(2):
                for tcn in range(2):
                    nc.gpsimd.indirect_dma_start(
                        out=xembs[s][:, tcn, :], out_offset=None, in_=emb.ap(),
                        in_offset=bass.IndirectOffsetOnAxis(ap=it[:, s, tcn:tcn + 1], axis=0))

            if STAGE < 2:
                continue
            # ---- x transposes -> cmpin kc 0..3 (feature-major cat, both seqs) ----
            cmpin = work.tile([128, 8, 512], f8, tag="cmpin", bufs=3, name=f"cmpin{b}")
            for s in range(2):
                for tcn in range(2):
                    for dc in range(4):
                        ptr = trT_ps(f"xT{b}_{s}{tcn}{dc}")
                        nc.tensor.matmul(ptr[:], lhsT=xembs[s][:, tcn, dc * 128:(dc + 1) * 128],
                                         rhs=ident8[:], start=True, stop=True)
                        dst = cmpin[:, dc, s * 256 + tcn * 128:s * 256 + (tcn + 1) * 128]
                        nc.vector.tensor_copy(dst, ptr[:])

            if STAGE < 3:
                continue
            # ---- self MLP (fp8 DRI) ----
            hmid = work.tile([128, 4, 512], f8, tag="mid", bufs=3, name=f"h1_{b}")
            mlp_fm(cmpin, 4, ws1, "b_s1", hmid, f"sm1_{b}")
            qb = work.tile([128, 4, 512], f8, tag="qpq", bufs=3, name=f"q_{b}")
            mlp_fm(hmid, 4, ws2, "b_s2", qb, f"sm2_{b}")

            if STAGE < 4:
                continue
            # ---- self attention, k-major (scores symmetric => rows are keys) ----
            for s in range(2):
                pS = [None, None]
                for kc in range(2):
                    p = attn_ps(f"S{b}_{s}{kc}")
                    nc.tensor.matmul(p[:], lhsT=identb[:], rhs=biasT[:, kc, :],
                                     start=True, stop=False, skip_group_check=True)
                    for kp in range(2):
                        nc.tensor.matmul(p[:], lhsT=qb[:, 2 * kp:2 * kp + 2, s * 256 + kc * 128:s * 256 + (kc + 1) * 128],
                                         rhs=qb[:, 2 * kp:2 * kp + 2, s * 256:(s + 1) * 256],
                                         start=False, stop=(kp == 1), perf_mode=DR,
                                         skip_group_check=True)
                    pS[kc] = p
                ET = work.tile([128, 2, 256], hf, tag="ET", bufs=3, name=f"ET{b}_{s}")
                attT = work.tile([128, 2, 256], f8, tag="attT", bufs=3, name=f"attT{b}_{s}")
                softmax_kmajor(ET, pS, [mcol[:, s, kc:kc + 1] for kc in range(2)],
                               attT, f"att{b}_{s}")
                # ctx feature-major -> cmpin[:, 4+dc, :] (DR over key chunks)
                for dc in range(4):
                    pm = attn_ps(f"ctxT{b}_{s}{dc}")
                    nc.tensor.matmul(pm[:], lhsT=xembs[s][:, :, dc * 128:(dc + 1) * 128],
                                     rhs=attT[:, 0:2, :], start=True, stop=True,
                                     perf_mode=DR)
                    copy_ps(dc % 2, cmpin[:, 4 + dc, s * 256:(s + 1) * 256], pm[:])
                if b == 0 and dbg and s == 0:
                    if "dbg_ET" in dbg:
                        stg = work.tile([128, 2, 256], f32, tag="tapET", bufs=1, name="tapET")
                        nc.vector.tensor_copy(stg[:], ET[:])
                        nc.sync.dma_start(dbg["dbg_ET"].ap(), stg[:])
                    if "dbg_attT" in dbg:
                        stg = work.tile([128, 2, 256], f32, tag="tapattT", bufs=1, name="tapattT")
                        nc.vector.tensor_copy(stg[:], attT[:])
                        nc.sync.dma_start(dbg["dbg_attT"].ap(), stg[:])

            if STAGE < 6:
                continue
            # ---- inter MLP (input = cmpin kc 0..7, K=1024) ----
            mlp_fm(cmpin, 8, wa1, "b_a1", hmid, f"im1_{b}")
            mlp_fm(hmid, 4, wa2, "b_a2", qb, f"im2_{b}")  # qb = [pq | hk]

            if STAGE < 7:
                continue
            # ---- inter attention, k-major both directions ----
            # h2pT[p, h] = exp(z[p,h] + m_p) / sum_p' : z chunks, pq stationary
            # p2hT[h, p] = exp(z^T[h,p] + m_h) / sum_h' : z^T chunks, hk stationary
            interT = {}
            for (nm, base, rlo, ms) in (("h2p", 0, 256, 0), ("p2h", 256, 0, 1)):
                pZ = [None, None]
                for jc in range(2):
                    p = attn_ps(f"z{nm}{b}_{jc}")
                    for kp in range(2):
                        nc.tensor.matmul(p[:], lhsT=qb[:, 2 * kp:2 * kp + 2, base + jc * 128:base + (jc + 1) * 128],
                                         rhs=qb[:, 2 * kp:2 * kp + 2, rlo:rlo + 256],
                                         start=(kp == 0), stop=(kp == 1), perf_mode=DR,
                                         skip_group_check=True)
                    pZ[jc] = p
                ET = work.tile([128, 2, 256], hf, tag="ET", bufs=3, name=f"ETz{b}_{nm}")
                dT = work.tile([128, 2, 256], f8, tag="attT", bufs=3, name=f"{nm}T{b}")
                softmax_kmajor(ET, pZ, [mcol[:, ms, jc:jc + 1] for jc in range(2)],
                               dT, f"{nm}{b}")
                interT[nm] = dT
            p2hT, h2pT = interT["p2h"], interT["h2p"]

            if STAGE < 9:
                continue
            # ---- Y = cat @ Wc1_bot (token-major out, feature-major input; DR) ----
            Yt = work.tile([128, 4, 512], f8, tag="Y", bufs=3, name=f"Y{b}")
            for s in range(2):
                for tcn in range(2):
                    pm = mm512_ps(f"Y{b}_{s}{tcn}")
                    for kp in range(4):
                        nc.tensor.matmul(pm[:], lhsT=cmpin[:, 2 * kp:2 * kp + 2, s * 256 + tcn * 128:s * 256 + (tcn + 1) * 128],
                                         rhs=wc1b[:, 2 * kp:2 * kp + 2, :],
                                         start=(kp == 0), stop=(kp == 3), perf_mode=DR)
                    copy_ps(tcn, Yt[:, s * 2 + tcn, :], pm[:])

            # ---- compare L1 (feature-major, both seqs; DR everywhere) ----
            cmp1 = work.tile([128, 4, 512], hf, tag="cmp1", bufs=3, name=f"cmp1_{b}")
            for nf in range(4):
                pm = mm512_ps(f"c1_{b}_nf{nf}")
                for kp in range(4):
                    nc.tensor.matmul(pm[:], lhsT=wc1t[:, kp, nf, :],
                                     rhs=cmpin[:, 2 * kp:2 * kp + 2, :],
                                     start=(kp == 0), stop=False, perf_mode=DRI)
                nc.tensor.matmul(pm[:, 0:256], lhsT=Yt[:, 2:4, nf * 128:(nf + 1) * 128],
                                 rhs=p2hT[:, 0:2, :], start=False, stop=False, perf_mode=DR)
                nc.tensor.matmul(pm[:, 256:512], lhsT=Yt[:, 0:2, nf * 128:(nf + 1) * 128],
                                 rhs=h2pT[:, 0:2, :], start=False, stop=True, perf_mode=DR)
                act_relu(nf % 2, cmp1[:, nf, :], pm[:], bias_ap("b_c1", nf))

            if STAGE < 10:
                continue
            # ---- compare L2 (token-major, fp16) + masked sum (DR fp8) ----
            for s in range(2):
                cmp2 = work.tile([128, 2, 512], f8, tag="cmp2", bufs=3, name=f"cmp2_{b}_{s}")
                for tcn in range(2):
                    pm = mm512_ps(f"c2_{b}_{s}{tcn}")
                    nc.tensor.matmul(pm[:], lhsT=ones[:], rhs=bc2row[:], start=True, stop=False)
                    for kc in range(4):
                        nc.tensor.matmul(pm[:], lhsT=cmp1[:, kc, s * 256 + tcn * 128:s * 256 + (tcn + 1) * 128],
                                         rhs=wc2[:, kc, :], start=False, stop=(kc == 3))
                    if tcn == 0:
                        nc.scalar.activation(cmp2[:, tcn, :], pm[:], AF.Relu)
                    else:
                        nc.vector.tensor_scalar(cmp2[:, tcn, :], pm[:], 0.0, None,
                                                op0=ALU.max)
                pa = ps.tile([1, 512], f32, space="PSUM", tag="mm512", bufs=3, name=f"sum{b}_{s}")
                for tcn in range(2):
                    nc.tensor.matmul(pa[:], lhsT=maskf[:, s, tcn, :], rhs=cmp2[:, tcn, :],
                                     start=(tcn == 0), stop=(tcn == 1))
                srow = work.tile([1, 512], f32, tag="sumrow", bufs=3, name=f"srow{b}_{s}")
                nc.vector.tensor_copy(srow[:], pa[:])
                nc.sync.dma_start(srows[s * BL + b:s * BL + b + 1, :].bitcast(f32), srow[:])

            if b == 0 and dbg:
                def tap(name, src_ap):
                    if name in dbg:
                        stg = work.tile(list(dbg[name].shape), f32, tag=f"tap_{name}", bufs=1, name=f"tap{name}")
                        nc.vector.tensor_copy(stg[:], src_ap)
                        nc.sync.dma_start(dbg[name].ap(), stg[:])
                tap("dbg_cmpin", cmpin[:])
                tap("dbg_q", qb[:])
                tap("dbg_p2hT", p2hT[:])
                tap("dbg_h2pT", h2pT[:])
                tap("dbg_Y", Yt[:])
                tap("dbg_cmp1", cmp1[:])

        # ---------------- aggregate MLP (all items at once, fp16) ----------------
        run_agg = (nitems == BL) and STAGE >= 11
        if run_agg:
            aggT = work.tile([128, 2, 4, BL], hf, tag="aggT", bufs=1, name="aggT")
            for dc in range(4):
                ptr = ps.tile([128, 2 * BL], f32r, space="PSUM", tag="trT", bufs=2,
                              name=f"aggTr{dc}")
                nc.tensor.matmul(ptr[:], lhsT=srows[:, dc * 128:(dc + 1) * 128],
                                 rhs=identr[0:2 * BL, 0:2 * BL], is_transpose=True,
                                 start=True, stop=True)
                for s in range(2):
                    nc.vector.tensor_copy(aggT[:, s, dc, :],
                                          ptr[:, s * BL:(s + 1) * BL].bitcast(f32))
            agg1 = work.tile([128, 4, BL], hf, tag="agg1", bufs=1, name="agg1")
            for nf in range(4):
                pm = attn_ps(f"g1_{nf}")
                for kc in range(8):
                    nc.tensor.matmul(pm[:, 0:BL], lhsT=wg1[:, kc, nf * 128:(nf + 1) * 128],
                                     rhs=aggT[:, kc // 4, kc % 4, :], start=(kc == 0), stop=(kc == 7))
                nc.scalar.activation(agg1[:, nf, :], pm[:, 0:BL], AF.Relu, bias=bias_ap("b_g1", nf))
            agg2 = work.tile([128, 4, BL], hf, tag="agg2", bufs=1, name="agg2")
            for nf in range(4):
                pm = attn_ps(f"g2_{nf}")
                for kc in range(4):
                    nc.tensor.matmul(pm[:, 0:BL], lhsT=wg2[:, kc, nf * 128:(nf + 1) * 128],
                                     rhs=agg1[:, kc, :], start=(kc == 0), stop=(kc == 3))
                nc.scalar.activation(agg2[:, nf, :], pm[:, 0:BL], AF.Relu, bias=bias_ap("b_g2", nf))
            po = attn_ps("po")
            for kc in range(4):
                nc.tensor.matmul(po[0:BL, 0:4], lhsT=agg2[:, kc, :], rhs=wo[:, kc, :],
                                 start=(kc == 0), stop=(kc == 3))
            osb = work.tile([BL, OUT], f32, tag="osb", bufs=1, name="osb")
            nc.vector.tensor_copy(osb[:], po[0:BL, 0:OUT])
            nc.sync.dma_start(out_d.ap(), osb[:])

        ps.release()
        work.release()
        const.release()

    nc.compile()
    return nc


def _get_program(debug_taps=()):
    key = tuple(n for n, _ in debug_taps)
    if key not in _PROG_CACHE:
        _PROG_CACHE[key] = _build_program(debug_taps)
    return _PROG_CACHE[key]


def kernel(prem_input, hypo_input, embed_W, dist_W,
           Ws1, bs1, Ws2, bs2, Wa1, ba1, Wa2, ba2,
           Wc1, bc1, Wc2, bc2, Wg1, bg1, Wg2, bg2, Wo,
           _debug_taps=(), _trace=False, _tmpdir=None):
    import concourse.mybir as mybir
    from concourse.bass_utils import run_bass_kernel_spmd

    nc = _get_program(_debug_taps)

    f32 = np.float32
    np_f8 = mybir.dt.np(mybir.dt.float8e4)

    def as_hf(a):
        return np.ascontiguousarray(np.asarray(a, f32).astype(np.float16))

    def as_f8(a):
        return np.ascontiguousarray(np.asarray(a, f32).astype(np_f8))

    def pack_km(a, dtype):
        """[K, 512] -> [128, K//128, 512]: partition p, chunk c <- row c*128+p."""
        W = np.asarray(a, f32).astype(dtype)
        K = W.shape[0]
        return np.ascontiguousarray(W.reshape(K // 128, 128, W.shape[1]).transpose(1, 0, 2))

    def as_dri(a):
        """fp8 weight [K, 512] -> DoubleRowSwInterleave stationary layout
        [128, K//256 pairs, 4 nf-chunks, 256]: per 128x128 k-tile pair the
        column pairs (A,B) are interleaved with columns reversed."""
        W = np.asarray(a, f32).astype(np_f8)
        K = W.shape[0]
        t = W.reshape(K // 128, 128, 4, 128)          # [kc, p, nf, m]
        rev = t[:, :, :, ::-1]
        out = np.empty((128, K // 256, 4, 256), np_f8)
        out[:, :, :, 0::2] = rev[0::2].transpose(1, 0, 2, 3)
        out[:, :, :, 1::2] = rev[1::2].transpose(1, 0, 2, 3)
        return np.ascontiguousarray(out)

    def pack_bias(*bs):
        # [128, 8, 4]: bias n at [:, n, c] = b[c*128 + p]
        return np.ascontiguousarray(
            np.stack([np.asarray(b, f32).reshape(4, 128).T for b in bs], axis=1))

    # transposed Toeplitz relative-distance bias (+ -30000 diagonal), fp16:
    # biasT[p, ic, q] = strip[255 + 128*ic + p - q], strip[255+d] = dW[clip d]
    dW = np.asarray(dist_W, f32).reshape(-1)
    strip = np.empty(2 * L - 1, f32)
    strip[:L - 1 - MAX_DIST] = dW[0]
    strip[L - 1 - MAX_DIST:L + MAX_DIST] = dW
    strip[L + MAX_DIST:] = dW[2 * MAX_DIST]
    strip[L - 1] = DIAG_VAL
    p_i = np.arange(128)[:, None]
    q_i = np.arange(256)[None, :]
    biasT = np.empty((128, 2, 256), np.float16)
    for ic in range(2):
        biasT[:, ic, :] = strip[255 + 128 * ic + p_i - q_i].astype(np.float16)

    # wo zero-padded [128, 4, 4]
    wo_h = np.zeros((128, 4, 4), np.float16)
    wo_h[:, :, :OUT] = np.asarray(Wo, f32).reshape(4, 128, OUT).transpose(1, 0, 2)

    Wc1f = np.asarray(Wc1, f32)
    common = {
        "emb": as_f8(embed_W),
        "w_s1": as_dri(Ws1), "w_s2": as_dri(Ws2),
        "w_a1": as_dri(Wa1), "w_a2": as_dri(Wa2),
        "w_c1t": as_dri(Wc1f[:2 * E]), "w_c1b": pack_km(Wc1f[2 * E:], np_f8),
        "w_c2": pack_km(Wc2, np.float16),
        "w_g1": pack_km(Wg1, np.float16), "w_g2": pack_km(Wg2, np.float16),
        "w_o": np.ascontiguousarray(wo_h),
        "biases": pack_bias(bs1, bs2, ba1, ba2, bc1, bc2, bg1, bg2),
        "bc2row": as_hf(np.asarray(bc2, f32).reshape(1, D)),
        "biasT": np.ascontiguousarray(biasT),
    }
    prem = np.ascontiguousarray(np.asarray(prem_input).reshape(B, L).astype(np.int32))
    hypo = np.ascontiguousarray(np.asarray(hypo_input).reshape(B, L).astype(np.int32))

    in_maps = []
    for c in range(NCORES):
        sl = slice(c * BL, (c + 1) * BL)
        tokc = np.stack([prem[sl], hypo[sl]], axis=0)  # [2, BL, L]
        in_maps.append({"tok": np.ascontiguousarray(tokc), **common})

    kwargs = {}
    if _trace:
        kwargs.update(trace=True, tmpdir=_tmpdir)
    res = run_bass_kernel_spmd(nc, in_maps, core_ids=list(range(NCORES)), **kwargs)
    out = np.concatenate([r["out"] for r in res.results], axis=0)
    if _debug_taps or _trace:
        return out, res
    return out


# revision 10
# speedup vs baseline: 1.1422x; 1.1422x over previous
"""Trainium2 Bass kernel for nn_BernoulliDecompAttModel (decomposable attention NLI model).

Contract: kernel(**inputs) takes the FULL unsharded inputs (as produced by
setup_inputs()) and returns the FULL [64, 3] float32 output. Internally the
batch (64) is sharded 8-ways across 8 NeuronCores (pure data parallel, all
weights replicated); each core runs an identical Bass/Tile program on its 8
batch items.

v2 (416us -> ?): key-major attention via score symmetry. Q.Q^T is symmetric
and z^T is the operand-swapped matmul, so every attention matrix is computed
directly with KEYS on partitions:
  - key masks become per-partition activation biases on the exp (no more
    ones[1,128] mask-inject matmuls),
  - the relative-distance bias enters the score PSUM as one identity matmul
    of a host-precomputed transposed-Toeplitz biasT tile,
  - softmax denominators = partition sums via ones_col[128,1] matmuls
    accumulated across both key chunks,
  - normalization happens in the natural layout: attT = ET * bcast(1/sum)
    with the reciprocal row broadcast across partitions on the idle GPSIMD
    engine (partition_broadcast), multiplied on DVE straight to fp8.
This deletes all is_transpose matmuls (attT/p2hT/h2pT/zT), all mask matmuls,
the f32r PSUM trick, and the madd/idxrow path. Weights are host-packed into
their exact SBUF layouts (single dense DMA each, biases packed in one
tensor, biasT Toeplitz built on host). A burst of identity warm-up matmuls
trips the HAM activity window during the initial weight/token DMA so real
matmuls start at 2.4 GHz. Scalar/vector work is split ~evenly.

fp8-e4m3 operands + DoubleRow / DoubleRowSwInterleave on all numerically
tolerant matmuls (self/inter MLPs, compare-L1 + Y, ctx, scores); fp16 on
cmp2 and the aggregate head. PSUM accumulation f32; softmax skips
max-subtraction (scores bounded, exp cannot overflow).

Layout conventions unchanged from v1: activations feature-major
[128 partitions = feature chunk, K/128 chunks, tokens on free dim]; prem and
hypo concatenated on the token axis for shared-weight MLPs (N=512).
"""

import numpy as np
import os

B, L, V, E, D, OUT = 64, 256, 50000, 512, 512, 3
NCORES = 8
BL = B // NCORES            # batch items per core
MAX_DIST = 11
MASK_VAL = -30000.0         # padded-key additive mask (exp() underflows to 0)
DIAG_VAL = -30000.0         # self-attention diagonal (fp16-safe; exp() -> 0)

_PROG_CACHE = {}


def _build_program(debug_taps=()):
    import concourse.bass as bass
    import concourse.bacc as bacc
    import concourse.mybir as mybir
    from concourse.tile import TileContext
    from concourse.masks import make_identity

    dt = mybir.dt
    f32, f32r, i32 = dt.float32, dt.float32r, dt.int32
    hf, f8 = dt.float16, dt.float8e4
    DR = mybir.MatmulPerfMode.DoubleRow
    DRI = mybir.MatmulPerfMode.DoubleRowSwInterleave
    AF = mybir.ActivationFunctionType
    ALU = mybir.AluOpType

    PB_MODE = os.environ.get('KPBCAST', 'gp')  # 'gp' gpsimd | 'pe' outer-product

    nc = bacc.Bacc("TRN2", target_bir_lowering=False, debug=True)

    # ---------------- DRAM I/O (all weights host-packed to SBUF layout) ----
    tok = nc.dram_tensor("tok", [2, BL, L], i32, kind="ExternalInput")
    emb = nc.dram_tensor("emb", [V, E], f8, kind="ExternalInput")
    w_s1 = nc.dram_tensor("w_s1", [128, E // 256, 4, 256], f8, kind="ExternalInput")
    w_s2 = nc.dram_tensor("w_s2", [128, D // 256, 4, 256], f8, kind="ExternalInput")
    w_a1 = nc.dram_tensor("w_a1", [128, 2 * E // 256, 4, 256], f8, kind="ExternalInput")
    w_a2 = nc.dram_tensor("w_a2", [128, D // 256, 4, 256], f8, kind="ExternalInput")
    w_c1t = nc.dram_tensor("w_c1t", [128, 2 * E // 256, 4, 256], f8, kind="ExternalInput")
    w_c1b = nc.dram_tensor("w_c1b", [128, 8, 512], f8, kind="ExternalInput")
    w_c2 = nc.dram_tensor("w_c2", [128, 4, 512], hf, kind="ExternalInput")
    w_g1 = nc.dram_tensor("w_g1", [128, 8, 512], hf, kind="ExternalInput")
    w_g2 = nc.dram_tensor("w_g2", [128, 4, 512], hf, kind="ExternalInput")
    w_o = nc.dram_tensor("w_o", [128, 4, 4], hf, kind="ExternalInput")
    biases = nc.dram_tensor("biases", [128, 8, 4], f32, kind="ExternalInput")
    bc2_d = nc.dram_tensor("bc2row", [1, D], hf, kind="ExternalInput")
    biasT_d = nc.dram_tensor("biasT", [128, 2, 256], hf, kind="ExternalInput")

    out_d = nc.dram_tensor("out", [BL, OUT], f32, kind="ExternalOutput")

    dbg = {}
    for name, shape in debug_taps:
        dbg[name] = nc.dram_tensor(name, shape, f32, kind="ExternalOutput")

    with TileContext(nc) as tc:
        const = tc.alloc_tile_pool(name="const", bufs=1)
        work = tc.alloc_tile_pool(name="work", bufs=2)
        ps = tc.alloc_tile_pool(name="ps", bufs=1, space="PSUM")

        def mm512_ps(name):
            return ps.tile([128, 512], f32, space="PSUM", tag="mm512", bufs=3, name=name)

        def attn_ps(name):
            return ps.tile([128, 256], f32, space="PSUM", tag="attn", bufs=5, name=name)

        def trT_ps(name):
            return ps.tile([128, 128], f32, space="PSUM", tag="attn", bufs=5, name=name)

        def rsum_ps(name):
            return ps.tile([1, 256], f32, space="PSUM", tag="attn", bufs=5, name=name)

        # ---------------- constants ----------------
        ident32 = const.tile([128, 128], f32, name="ident32")
        make_identity(nc, ident32[:])
        ident8 = const.tile([128, 128], f8, name="ident8")
        nc.vector.tensor_copy(ident8[:], ident32[:])
        identb = const.tile([128, 128], hf, name="identb")
        nc.vector.tensor_copy(identb[:], ident32[:])
        identr = const.tile([128, 128], f32r, name="identr")
        nc.vector.tensor_copy(identr[:], ident32[:])

        ones = const.tile([1, 128], hf, name="ones")        # K=1 broadcast lhsT
        nc.vector.memset(ones[:], 1.0)
        ones_col = const.tile([128, 1], hf, name="ones_col")  # partition-sum lhsT
        nc.vector.memset(ones_col[:], 1.0)

        # HAM warm-up: ~30 back-to-back idents trip the PE activity window
        # while token/weight DMAs stream, so item-0 matmuls run at 2.4 GHz.
        for wi in range(30):
            pw = ps.tile([128, 128], f32, space="PSUM", tag="attn", bufs=5,
                         name=f"warm{wi}")
            nc.tensor.matmul(pw[:], lhsT=ident8[:], rhs=ident8[:],
                             start=True, stop=True)

        # ---------------- weights (single dense DMA each) ----------------
        def load_const(dram, shape, dtype, name):
            t = const.tile(shape, dtype, name=name)
            nc.sync.dma_start(t[:], dram.ap())
            return t

        ws1 = load_const(w_s1, [128, 2, 4, 256], f8, "ws1")
        ws2 = load_const(w_s2, [128, 2, 4, 256], f8, "ws2")
        wa1 = load_const(w_a1, [128, 4, 4, 256], f8, "wa1")
        wa2 = load_const(w_a2, [128, 2, 4, 256], f8, "wa2")
        wc1t = load_const(w_c1t, [128, 4, 4, 256], f8, "wc1t")
        wc1b = load_const(w_c1b, [128, 8, 512], f8, "wc1b")
        wc2 = load_const(w_c2, [128, 4, 512], hf, "wc2")
        wg1 = load_const(w_g1, [128, 8, 512], hf, "wg1")
        wg2 = load_const(w_g2, [128, 4, 512], hf, "wg2")
        wo = load_const(w_o, [128, 4, 4], hf, "wo")
        bsb = load_const(biases, [128, 8, 4], f32, "bsb")
        BIDX = {n: i for i, n in enumerate(
            ["b_s1", "b_s2", "b_a1", "b_a2", "b_c1", "b_c2", "b_g1", "b_g2"])}

        def bias_ap(n, nf):
            return bsb[:, BIDX[n], nf:nf + 1]

        bc2row = load_const(bc2_d, [1, D], hf, "bc2row")
        biasT = load_const(biasT_d, [128, 2, 256], hf, "biasT")

        srows = const.tile([2 * BL, 512], f32r, name="srows")

        # ---------------- helpers ----------------
        def act_relu(engine_pick, dst_ap, src_ap, bias):
            """relu(src + bias) -> dst; alternate scalar ACT / DVE tensor_scalar."""
            if engine_pick == 0:
                nc.scalar.activation(dst_ap, src_ap, AF.Relu, bias=bias)
            else:
                nc.vector.tensor_scalar(dst_ap, src_ap, bias, 0.0,
                                        op0=ALU.add, op1=ALU.max)

        def copy_ps(engine_pick, dst_ap, src_ap):
            if engine_pick == 0:
                nc.scalar.copy(dst_ap, src_ap)
            else:
                nc.vector.tensor_copy(dst_ap, src_ap)

        def mlp_fm(src, nkc, w, bname, dst, name):
            """feature-major MLP layer: dst[:,nf,:] = relu(w.T @ src + bias).
            w in the DRI interleaved layout [128, pairs, 4, 256]."""
            npair = nkc // 2
            for nf in range(4):
                pm = mm512_ps(f"{name}_nf{nf}")
                for i in range(npair):
                    nc.tensor.matmul(pm[:], lhsT=w[:, i, nf, :],
                                     rhs=src[:, 2 * i:2 * i + 2, :],
                                     start=(i == 0), stop=(i == npair - 1),
                                     perf_mode=DRI)
                act_relu(nf % 2, dst[:, nf, :], pm[:], bias_ap(bname, nf))

        def softmax_kmajor(ET, pS_by_kc, mcol_by_kc, dstT, name):
            """k-major masked softmax + normalize.

            ET[:, kc, :] = exp(pS_kc + maskcol_kc)   (keys on partitions)
            rowsum[q] = sum_k ET[k, q] via ones_col matmuls (both chunks)
            dstT[:, kc, :] = ET[:, kc, :] * bcast(1/rowsum)  -> fp8
            """
            for kc in range(2):
                nc.scalar.activation(ET[:, kc, :], pS_by_kc[kc][:], AF.Exp,
                                     bias=mcol_by_kc[kc])
            prs = rsum_ps(f"rs_{name}")
            for kc in range(2):
                nc.tensor.matmul(prs[:], lhsT=ones_col[:], rhs=ET[:, kc, :],
                                 start=(kc == 0), stop=(kc == 1))
            rec32 = work.tile([1, 256], f32, tag="rec32", bufs=4, name=f"rec32_{name}")
            nc.vector.reciprocal_approx_fast(rec32[:], prs[:])
            rec16 = work.tile([1, 256], hf, tag="rec16", bufs=4, name=f"rec16_{name}")
            nc.vector.tensor_copy(rec16[:], rec32[:])
            if PB_MODE == 'gp':
                recb = work.tile([128, 256], hf, tag="recb", bufs=3, name=f"recb_{name}")
                nc.gpsimd.partition_broadcast(recb[:], rec16[:], 128)
                for kc in range(2):
                    nc.vector.tensor_tensor(dstT[:, kc, :], ET[:, kc, :], recb[:],
                                            op=ALU.mult)
            else:
                prb = attn_ps(f"rb_{name}")
                nc.tensor.matmul(prb[:], lhsT=ones[:], rhs=rec16[:],
                                 start=True, stop=True)
                for kc in range(2):
                    nc.vector.tensor_tensor(dstT[:, kc, :], ET[:, kc, :], prb[:],
                                            op=ALU.mult)

        # ---------------- per-item pipeline ----------------
        nitems = int(os.environ.get('KITEMS', BL))
        STAGE = int(os.environ.get('KSTAGE', 99))
        for b in range(nitems):
            # ---- indices, masks ----
            it = work.tile([128, 2, 2], i32, tag="it", bufs=3, name=f"it{b}")
            for s in range(2):
                nc.sync.dma_start(it[:, s, :], bass.AP(tok, b * L + s * BL * L, [[1, 128], [128, 2]]))
            maskf = work.tile([128, 2, 2, 1], f8, tag="maskf", bufs=3, name=f"maskf{b}")
            nc.vector.tensor_scalar(maskf[:], it[:], 0, None, op0=ALU.not_equal)
            # additive key-mask column: -30000 where token == PAD
            mcol = work.tile([128, 2, 2], f32, tag="mcol", bufs=3, name=f"mcol{b}")
            nc.vector.tensor_scalar(mcol[:], it[:], 0, MASK_VAL,
                                    op0=ALU.is_equal, op1=ALU.mult)

            # ---- embedding gather (token-major, fp8 direct) ----
            xembs = [work.tile([128, 2, E], f8, tag=f"xembs{s}", bufs=3, name=f"xembs{b}_{s}")
                     for s in range(2)]
            for s in range(2):
                for tcn in range(2):
                    nc.gpsimd.indirect_dma_start(
                        out=xembs[s][:, tcn, :], out_offset=None, in_=emb.ap(),
                        in_offset=bass.IndirectOffsetOnAxis(ap=it[:, s, tcn:tcn + 1], axis=0))

            if STAGE < 2:
                continue
            # ---- x transposes -> cmpin kc 0..3 (feature-major cat, both seqs) ----
            cmpin = work.tile([128, 8, 512], f8, tag="cmpin", bufs=3, name=f"cmpin{b}")
            for s in range(2):
                for tcn in range(2):
                    for dc in range(4):
                        ptr = trT_ps(f"xT{b}_{s}{tcn}{dc}")
                        nc.tensor.matmul(ptr[:], lhsT=xembs[s][:, tcn, dc * 128:(dc + 1) * 128],
                                         rhs=ident8[:], start=True, stop=True)
                        dst = cmpin[:, dc, s * 256 + tcn * 128:s * 256 + (tcn + 1) * 128]
                        copy_ps((dc + tcn) % 2, dst, ptr[:])

            if STAGE < 3:
                continue
            # ---- self MLP (fp8 DRI) ----
            hmid = work.tile([128, 4, 512], f8, tag="mid", bufs=3, name=f"h1_{b}")
            mlp_fm(cmpin, 4, ws1, "b_s1", hmid, f"sm1_{b}")
            qb = work.tile([128, 4, 512], f8, tag="qpq", bufs=3, name=f"q_{b}")
            mlp_fm(hmid, 4, ws2, "b_s2", qb, f"sm2_{b}")

            if STAGE < 4:
                continue
            # ---- self attention, k-major (scores symmetric => rows are keys) ----
            for s in range(2):
                pS = [None, None]
                for kc in range(2):
                    p = attn_ps(f"S{b}_{s}{kc}")
                    nc.tensor.matmul(p[:], lhsT=identb[:], rhs=biasT[:, kc, :],
                                     start=True, stop=False, skip_group_check=True)
                    for kp in range(2):
                        nc.tensor.matmul(p[:], lhsT=qb[:, 2 * kp:2 * kp + 2, s * 256 + kc * 128:s * 256 + (kc + 1) * 128],
                                         rhs=qb[:, 2 * kp:2 * kp + 2, s * 256:(s + 1) * 256],
                                         start=False, stop=(kp == 1), perf_mode=DR,
                                         skip_group_check=True)
                    pS[kc] = p
                ET = work.tile([128, 2, 256], hf, tag="ET", bufs=3, name=f"ET{b}_{s}")
                attT = work.tile([128, 2, 256], f8, tag="attT", bufs=3, name=f"attT{b}_{s}")
                softmax_kmajor(ET, pS, [mcol[:, s, kc:kc + 1] for kc in range(2)],
                               attT, f"att{b}_{s}")
                # ctx feature-major -> cmpin[:, 4+dc, :] (DR over key chunks)
                for dc in range(4):
                    pm = attn_ps(f"ctxT{b}_{s}{dc}")
                    nc.tensor.matmul(pm[:], lhsT=xembs[s][:, :, dc * 128:(dc + 1) * 128],
                                     rhs=attT[:, 0:2, :], start=True, stop=True,
                                     perf_mode=DR)
                    copy_ps(dc % 2, cmpin[:, 4 + dc, s * 256:(s + 1) * 256], pm[:])
                if b == 0 and dbg and s == 0:
                    if "dbg_ET" in dbg:
                        stg = work.tile([128, 2, 256], f32, tag="tapET", bufs=1, name="tapET")
                        nc.vector.tensor_copy(stg[:], ET[:])
                        nc.sync.dma_start(dbg["dbg_ET"].ap(), stg[:])
                    if "dbg_attT" in dbg:
                        stg = work.tile([128, 2, 256], f32, tag="tapattT", bufs=1, name="tapattT")
                        nc.vector.tensor_copy(stg[:], attT[:])
                        nc.sync.dma_start(dbg["dbg_attT"].ap(), stg[:])

            if STAGE < 6:
                continue
            # ---- inter MLP (input = cmpin kc 0..7, K=1024) ----
            mlp_fm(cmpin, 8, wa1, "b_a1", hmid, f"im1_{b}")
            mlp_fm(hmid, 4, wa2, "b_a2", qb, f"im2_{b}")  # qb = [pq | hk]

            if STAGE < 7:
                continue
            # ---- inter attention, k-major both directions ----
            # h2pT[p, h] = exp(z[p,h] + m_p) / sum_p' : z chunks, pq stationary
            # p2hT[h, p] = exp(z^T[h,p] + m_h) / sum_h' : z^T chunks, hk stationary
            interT = {}
            for (nm, base, rlo, ms) in (("h2p", 0, 256, 0), ("p2h", 256, 0, 1)):
                pZ = [None, None]
                for jc in range(2):
                    p = attn_ps(f"z{nm}{b}_{jc}")
                    for kp in range(2):
                        nc.tensor.matmul(p[:], lhsT=qb[:, 2 * kp:2 * kp + 2, base + jc * 128:base + (jc + 1) * 128],
                                         rhs=qb[:, 2 * kp:2 * kp + 2, rlo:rlo + 256],
                                         start=(kp == 0), stop=(kp == 1), perf_mode=DR,
                                         skip_group_check=True)
                    pZ[jc] = p
                ET = work.tile([128, 2, 256], hf, tag="ET", bufs=3, name=f"ETz{b}_{nm}")
                dT = work.tile([128, 2, 256], f8, tag="attT", bufs=3, name=f"{nm}T{b}")
                softmax_kmajor(ET, pZ, [mcol[:, ms, jc:jc + 1] for jc in range(2)],
                               dT, f"{nm}{b}")
                interT[nm] = dT
            p2hT, h2pT = interT["p2h"], interT["h2p"]

            if STAGE < 9:
                continue
            # ---- Y = cat @ Wc1_bot (token-major out, feature-major input; DR) ----
            Yt = work.tile([128, 4, 512], f8, tag="Y", bufs=3, name=f"Y{b}")
            for s in range(2):
                for tcn in range(2):
                    pm = mm512_ps(f"Y{b}_{s}{tcn}")
                    for kp in range(4):
                        nc.tensor.matmul(pm[:], lhsT=cmpin[:, 2 * kp:2 * kp + 2, s * 256 + tcn * 128:s * 256 + (tcn + 1) * 128],
                                         rhs=wc1b[:, 2 * kp:2 * kp + 2, :],
                                         start=(kp == 0), stop=(kp == 3), perf_mode=DR)
                    copy_ps(tcn, Yt[:, s * 2 + tcn, :], pm[:])

            # ---- compare L1 (feature-major, both seqs; DR everywhere) ----
            cmp1 = work.tile([128, 4, 512], hf, tag="cmp1", bufs=3, name=f"cmp1_{b}")
            for nf in range(4):
                pm = mm512_ps(f"c1_{b}_nf{nf}")
                for kp in range(4):
                    nc.tensor.matmul(pm[:], lhsT=wc1t[:, kp, nf, :],
                                     rhs=cmpin[:, 2 * kp:2 * kp + 2, :],
                                     start=(kp == 0), stop=False, perf_mode=DRI)
                nc.tensor.matmul(pm[:, 0:256], lhsT=Yt[:, 2:4, nf * 128:(nf + 1) * 128],
                                 rhs=p2hT[:, 0:2, :], start=False, stop=False, perf_mode=DR)
                nc.tensor.matmul(pm[:, 256:512], lhsT=Yt[:, 0:2, nf * 128:(nf + 1) * 128],
                                 rhs=h2pT[:, 0:2, :], start=False, stop=True, perf_mode=DR)
                act_relu(nf % 2, cmp1[:, nf, :], pm[:], bias_ap("b_c1", nf))

            if STAGE < 10:
                continue
            # ---- compare L2 (token-major, fp16) + masked sum (DR fp8) ----
            for s in range(2):
                cmp2 = work.tile([128, 2, 512], f8, tag="cmp2", bufs=3, name=f"cmp2_{b}_{s}")
                for tcn in range(2):
                    pm = mm512_ps(f"c2_{b}_{s}{tcn}")
                    nc.tensor.matmul(pm[:], lhsT=ones[:], rhs=bc2row[:], start=True, stop=False)
                    for kc in range(4):
                        nc.tensor.matmul(pm[:], lhsT=cmp1[:, kc, s * 256 + tcn * 128:s * 256 + (tcn + 1) * 128],
                                         rhs=wc2[:, kc, :], start=False, stop=(kc == 3))
                    if tcn == 0:
                        nc.scalar.activation(cmp2[:, tcn, :], pm[:], AF.Relu)
                    else:
                        nc.vector.tensor_scalar(cmp2[:, tcn, :], pm[:], 0.0, None,
                                                op0=ALU.max)
                pa = ps.tile([1, 512], f32, space="PSUM", tag="mm512", bufs=3, name=f"sum{b}_{s}")
                for tcn in range(2):
                    nc.tensor.matmul(pa[:], lhsT=maskf[:, s, tcn, :], rhs=cmp2[:, tcn, :],
                                     start=(tcn == 0), stop=(tcn == 1))
                srow = work.tile([1, 512], f32, tag="sumrow", bufs=3, name=f"srow{b}_{s}")
                nc.vector.tensor_copy(srow[:], pa[:])
                nc.sync.dma_start(srows[s * BL + b:s * BL + b + 1, :].bitcast(f32), srow[:])

            if b == 0 and dbg:
                def tap(name, src_ap):
                    if name in dbg:
                        stg = work.tile(list(dbg[name].shape), f32, tag=f"tap_{name}", bufs=1, name=f"tap{name}")
                        nc.vector.tensor_copy(stg[:], src_ap)
                        nc.sync.dma_start(dbg[name].ap(), stg[:])
                tap("dbg_cmpin", cmpin[:])
                tap("dbg_q", qb[:])
                tap("dbg_p2hT", p2hT[:])
                tap("dbg_h2pT", h2pT[:])
                tap("dbg_Y", Yt[:])
                tap("dbg_cmp1", cmp1[:])

        # ---------------- aggregate MLP (all items at once, fp16) ----------------
        run_agg = (nitems == BL) and STAGE >= 11
        if run_agg:
            aggT = work.tile([128, 2, 4, BL], hf, tag="aggT", bufs=1, name="aggT")
            for dc in range(4):
                ptr = ps.tile([128, 2 * BL], f32r, space="PSUM", tag="attn", bufs=5,
                              name=f"aggTr{dc}")
                nc.tensor.matmul(ptr[:], lhsT=srows[:, dc * 128:(dc + 1) * 128],
                                 rhs=identr[0:2 * BL, 0:2 * BL], is_transpose=True,
                                 start=True, stop=True)
                for s in range(2):
                    nc.vector.tensor_copy(aggT[:, s, dc, :],
                                          ptr[:, s * BL:(s + 1) * BL].bitcast(f32))
            agg1 = work.tile([128, 4, BL], hf, tag="agg1", bufs=1, name="agg1")
            for nf in range(4):
                pm = attn_ps(f"g1_{nf}")
                for kc in range(8):
                    nc.tensor.matmul(pm[:, 0:BL], lhsT=wg1[:, kc, nf * 128:(nf + 1) * 128],
                                     rhs=aggT[:, kc // 4, kc % 4, :], start=(kc == 0), stop=(kc == 7))
                nc.scalar.activation(agg1[:, nf, :], pm[:, 0:BL], AF.Relu, bias=bias_ap("b_g1", nf))
            agg2 = work.tile([128, 4, BL], hf, tag="agg2", bufs=1, name="agg2")
            for nf in range(4):
                pm = attn_ps(f"g2_{nf}")
                for kc in range(4):
                    nc.tensor.matmul(pm[:, 0:BL], lhsT=wg2[:, kc, nf * 128:(nf + 1) * 128],
                                     rhs=agg1[:, kc, :], start=(kc == 0), stop=(kc == 3))
                nc.scalar.activation(agg2[:, nf, :], pm[:, 0:BL], AF.Relu, bias=bias_ap("b_g2", nf))
            po = attn_ps("po")
            for kc in range(4):
                nc.tensor.matmul(po[0:BL, 0:4], lhsT=agg2[:, kc, :], rhs=wo[:, kc, :],
                                 start=(kc == 0), stop=(kc == 3))
            osb = work.tile([BL, OUT], f32, tag="osb", bufs=1, name="osb")
            nc.vector.tensor_copy(osb[:], po[0:BL, 0:OUT])
            nc.sync.dma_start(out_d.ap(), osb[:])

        ps.release()
        work.release()
        const.release()

    nc.compile()
    return nc


def _get_program(debug_taps=()):
    key = tuple(n for n, _ in debug_taps)
    if key not in _PROG_CACHE:
        _PROG_CACHE[key] = _build_program(debug_taps)
    return _PROG_CACHE[key]


def kernel(prem_input, hypo_input, embed_W, dist_W,
           Ws1, bs1, Ws2, bs2, Wa1, ba1, Wa2, ba2,
           Wc1, bc1, Wc2, bc2, Wg1, bg1, Wg2, bg2, Wo,
           _debug_taps=(), _trace=False, _tmpdir=None):
    import concourse.mybir as mybir
    from concourse.bass_utils import run_bass_kernel_spmd

    nc = _get_program(_debug_taps)

    f32 = np.float32
    np_f8 = mybir.dt.np(mybir.dt.float8e4)

    def as_hf(a):
        return np.ascontiguousarray(np.asarray(a, f32).astype(np.float16))

    def as_f8(a):
        return np.ascontiguousarray(np.asarray(a, f32).astype(np_f8))

    def pack_km(a, dtype):
        """[K, 512] -> [128, K//128, 512]: partition p, chunk c <- row c*128+p."""
        W = np.asarray(a, f32).astype(dtype)
        K = W.shape[0]
        return np.ascontiguousarray(W.reshape(K // 128, 128, W.shape[1]).transpose(1, 0, 2))

    def as_dri(a):
        """fp8 weight [K, 512] -> DoubleRowSwInterleave stationary layout
        [128, K//256 pairs, 4 nf-chunks, 256]: per 128x128 k-tile pair the
        column pairs (A,B) are interleaved with columns reversed."""
        W = np.asarray(a, f32).astype(np_f8)
        K = W.shape[0]
        t = W.reshape(K // 128, 128, 4, 128)          # [kc, p, nf, m]
        rev = t[:, :, :, ::-1]
        out = np.empty((128, K // 256, 4, 256), np_f8)
        out[:, :, :, 0::2] = rev[0::2].transpose(1, 0, 2, 3)
        out[:, :, :, 1::2] = rev[1::2].transpose(1, 0, 2, 3)
        return np.ascontiguousarray(out)

    def pack_bias(*bs):
        # [128, 8, 4]: bias n at [:, n, c] = b[c*128 + p]
        return np.ascontiguousarray(
            np.stack([np.asarray(b, f32).reshape(4, 128).T for b in bs], axis=1))

    # transposed Toeplitz relative-distance bias (+ -30000 diagonal), fp16:
    # biasT[p, ic, q] = strip[255 + 128*ic + p - q], strip[255+d] = dW[clip d]
    dW = np.asarray(dist_W, f32).reshape(-1)
    strip = np.empty(2 * L - 1, f32)
    strip[:L - 1 - MAX_DIST] = dW[0]
    strip[L - 1 - MAX_DIST:L + MAX_DIST] = dW
    strip[L + MAX_DIST:] = dW[2 * MAX_DIST]
    strip[L - 1] = DIAG_VAL
    p_i = np.arange(128)[:, None]
    q_i = np.arange(256)[None, :]
    biasT = np.empty((128, 2, 256), np.float16)
    for ic in range(2):
        biasT[:, ic, :] = strip[255 + 128 * ic + p_i - q_i].astype(np.float16)

    # wo zero-padded [128, 4, 4]
    wo_h = np.zeros((128, 4, 4), np.float16)
    wo_h[:, :, :OUT] = np.asarray(Wo, f32).reshape(4, 128, OUT).transpose(1, 0, 2)

    Wc1f = np.asarray(Wc1, f32)
    common = {
        "emb": as_f8(embed_W),
        "w_s1": as_dri(Ws1), "w_s2": as_dri(Ws2),
        "w_a1": as_dri(Wa1), "w_a2": as_dri(Wa2),
        "w_c1t": as_dri(Wc1f[:2 * E]), "w_c1b": pack_km(Wc1f[2 * E:], np_f8),
        "w_c2": pack_km(Wc2, np.float16),
        "w_g1": pack_km(Wg1, np.float16), "w_g2": pack_km(Wg2, np.float16),
        "w_o": np.ascontiguousarray(wo_h),
        "biases": pack_bias(bs1, bs2, ba1, ba2, bc1, bc2, bg1, bg2),
        "bc2row": as_hf(np.asarray(bc2, f32).reshape(1, D)),
        "biasT": np.ascontiguousarray(biasT),
    }
    prem = np.ascontiguousarray(np.asarray(prem_input).reshape(B, L).astype(np.int32))
    hypo = np.ascontiguousarray(np.asarray(hypo_input).reshape(B, L).astype(np.int32))

    in_maps = []
    for c in range(NCORES):
        sl = slice(c * BL, (c + 1) * BL)
        tokc = np.stack([prem[sl], hypo[sl]], axis=0)  # [2, BL, L]
        in_maps.append({"tok": np.ascontiguousarray(tokc), **common})

    kwargs = {}
    if _trace:
        kwargs.update(trace=True, tmpdir=_tmpdir)
    res = run_bass_kernel_spmd(nc, in_maps, core_ids=list(range(NCORES)), **kwargs)
    out = np.concatenate([r["out"] for r in res.results], axis=0)
    if _debug_taps or _trace:
        return out, res
    return out


# revision 11
# speedup vs baseline: 1.2584x; 1.1018x over previous
"""Trainium2 Bass kernel for nn_BernoulliDecompAttModel (decomposable attention NLI model).

Contract: kernel(**inputs) takes the FULL unsharded inputs (as produced by
setup_inputs()) and returns the FULL [64, 3] float32 output. Internally the
batch (64) is sharded 8-ways across 8 NeuronCores (pure data parallel, all
weights replicated); each core runs an identical Bass/Tile program on its 8
batch items.

v3 (416us -> 394us -> ?): software-pipelined 3-stage schedule. Each item's
work is split into A (gather/xT/self-MLP/self-attention softmax), B (ctx,
inter MLP, inter attention softmaxes) and C (Y, compare L1/L2, masked sum),
emitted A(b); B(b-1); C(b-2) so every softmax's engine chain (exp -> ones-
matmul row sum -> fast reciprocal -> outer-product broadcast -> DVE mult)
completes behind ~10us of other items' PE work instead of stalling the PE.
Token DMAs + embedding gathers for all items are hoisted into a prologue
(ahead of the weight DMAs in queue order), and ~30 identity warm-up matmuls
trip the HAM activity window so real matmuls start at 2.4 GHz.

Attention is computed k-major via score symmetry (Q.Q^T symmetric, z^T =
swapped matmul): no transpose matmuls, no mask-inject matmuls (key masks are
per-partition activation biases on the exp), relative-distance bias enters
score PSUM as one identity matmul of a host-precomputed transposed-Toeplitz
tile. Normalization multiplies by an outer-product broadcast (ones[1,128]
(x) rec16[1,256] on the PE) of the approx-reciprocal row.

fp8-e4m3 + DoubleRow(SwInterleave) on all numerically tolerant matmuls,
fp16 cmp2 + aggregate head (fp8 there breaks the 2e-2 gate; verified with a
numpy pipeline sim on the real data, sim matches HW to 3 digits). PSUM f32;
softmax skips max-subtraction (scores bounded).
"""

import numpy as np
import os

B, L, V, E, D, OUT = 64, 256, 50000, 512, 512, 3
NCORES = 8
BL = B // NCORES            # batch items per core
MAX_DIST = 11
MASK_VAL = -30000.0         # padded-key additive mask (exp() underflows to 0)
DIAG_VAL = -30000.0         # self-attention diagonal (fp16-safe; exp() -> 0)

_PROG_CACHE = {}


def _build_program(debug_taps=()):
    import concourse.bass as bass
    import concourse.bacc as bacc
    import concourse.mybir as mybir
    from concourse.tile import TileContext
    from concourse.masks import make_identity

    dt = mybir.dt
    f32, f32r, i32 = dt.float32, dt.float32r, dt.int32
    hf, f8 = dt.float16, dt.float8e4
    DR = mybir.MatmulPerfMode.DoubleRow
    DRI = mybir.MatmulPerfMode.DoubleRowSwInterleave
    AF = mybir.ActivationFunctionType
    ALU = mybir.AluOpType

    nc = bacc.Bacc("TRN2", target_bir_lowering=False, debug=True)

    # ---------------- DRAM I/O (all weights host-packed to SBUF layout) ----
    tok = nc.dram_tensor("tok", [2, BL, L], i32, kind="ExternalInput")
    emb = nc.dram_tensor("emb", [V, E], f8, kind="ExternalInput")
    w_s1 = nc.dram_tensor("w_s1", [128, E // 256, 4, 256], f8, kind="ExternalInput")
    w_s2 = nc.dram_tensor("w_s2", [128, D // 256, 4, 256], f8, kind="ExternalInput")
    w_a1 = nc.dram_tensor("w_a1", [128, 2 * E // 256, 4, 256], f8, kind="ExternalInput")
    w_a2 = nc.dram_tensor("w_a2", [128, D // 256, 4, 256], f8, kind="ExternalInput")
    w_c1t = nc.dram_tensor("w_c1t", [128, 2 * E // 256, 4, 256], f8, kind="ExternalInput")
    w_c1b = nc.dram_tensor("w_c1b", [128, 8, 512], f8, kind="ExternalInput")
    w_c2 = nc.dram_tensor("w_c2", [128, 4, 512], hf, kind="ExternalInput")
    w_g1 = nc.dram_tensor("w_g1", [128, 8, 512], hf, kind="ExternalInput")
    w_g2 = nc.dram_tensor("w_g2", [128, 4, 512], hf, kind="ExternalInput")
    w_o = nc.dram_tensor("w_o", [128, 4, 4], hf, kind="ExternalInput")
    biases = nc.dram_tensor("biases", [128, 8, 4], f32, kind="ExternalInput")
    bc2_d = nc.dram_tensor("bc2row", [1, D], hf, kind="ExternalInput")
    biasT_d = nc.dram_tensor("biasT", [128, 2, 256], hf, kind="ExternalInput")

    out_d = nc.dram_tensor("out", [BL, OUT], f32, kind="ExternalOutput")

    dbg = {}
    for name, shape in debug_taps:
        dbg[name] = nc.dram_tensor(name, shape, f32, kind="ExternalOutput")

    with TileContext(nc) as tc:
        const = tc.alloc_tile_pool(name="const", bufs=1)
        work = tc.alloc_tile_pool(name="work", bufs=2)
        ps = tc.alloc_tile_pool(name="ps", bufs=1, space="PSUM")

        def mm512_ps(name):
            return ps.tile([128, 512], f32, space="PSUM", tag="mm512", bufs=3, name=name)

        def attn_ps(name):
            return ps.tile([128, 256], f32, space="PSUM", tag="attn", bufs=5, name=name)

        def trT_ps(name):
            return ps.tile([128, 128], f32, space="PSUM", tag="attn", bufs=5, name=name)

        def rsum_ps(name):
            return ps.tile([1, 256], f32, space="PSUM", tag="attn", bufs=5, name=name)

        # ---------------- constants ----------------
        ident32 = const.tile([128, 128], f32, name="ident32")
        make_identity(nc, ident32[:])
        ident8 = const.tile([128, 128], f8, name="ident8")
        nc.vector.tensor_copy(ident8[:], ident32[:])
        identb = const.tile([128, 128], hf, name="identb")
        nc.vector.tensor_copy(identb[:], ident32[:])
        identr = const.tile([128, 128], f32r, name="identr")
        nc.vector.tensor_copy(identr[:], ident32[:])

        ones = const.tile([1, 128], hf, name="ones")        # K=1 broadcast lhsT
        nc.vector.memset(ones[:], 1.0)
        ones_col = const.tile([128, 1], hf, name="ones_col")  # partition-sum lhsT
        nc.vector.memset(ones_col[:], 1.0)

        nitems = int(os.environ.get('KITEMS', BL))

        # ---------------- prologue: tokens + masks + gathers for all items --
        its, maskfs, mcols, xembs = [], [], [], []
        for b in range(nitems):
            it = work.tile([128, 2, 2], i32, tag=f"it{b}", bufs=1, name=f"it{b}")
            for s in range(2):
                nc.sync.dma_start(it[:, s, :], bass.AP(tok, b * L + s * BL * L, [[1, 128], [128, 2]]))
            its.append(it)
        for b in range(nitems):
            mf = work.tile([128, 2, 2, 1], f8, tag=f"maskf{b}", bufs=1, name=f"maskf{b}")
            nc.vector.tensor_scalar(mf[:], its[b][:], 0, None, op0=ALU.not_equal)
            mc = work.tile([128, 2, 2], f32, tag=f"mcol{b}", bufs=1, name=f"mcol{b}")
            nc.vector.tensor_scalar(mc[:], its[b][:], 0, MASK_VAL,
                                    op0=ALU.is_equal, op1=ALU.mult)
            maskfs.append(mf)
            mcols.append(mc)
        for b in range(nitems):
            xe = [work.tile([128, 2, E], f8, tag=f"xemb{b}_{s}", bufs=1, name=f"xemb{b}_{s}")
                  for s in range(2)]
            for s in range(2):
                for tcn in range(2):
                    nc.gpsimd.indirect_dma_start(
                        out=xe[s][:, tcn, :], out_offset=None, in_=emb.ap(),
                        in_offset=bass.IndirectOffsetOnAxis(ap=its[b][:, s, tcn:tcn + 1], axis=0))
            xembs.append(xe)

        # HAM warm-up: identity matmuls trip the PE activity window while the
        # token/weight DMAs stream, so item-0 matmuls run at 2.4 GHz.
        for wi in range(30):
            pw = ps.tile([128, 128], f32, space="PSUM", tag="attn", bufs=5,
                         name=f"warm{wi}")
            nc.tensor.matmul(pw[:], lhsT=ident8[:], rhs=ident8[:],
                             start=True, stop=True)

        # ---------------- weights (single dense DMA each) ----------------
        def load_const(dram, shape, dtype, name):
            t = const.tile(shape, dtype, name=name)
            nc.sync.dma_start(t[:], dram.ap())
            return t

        ws1 = load_const(w_s1, [128, 2, 4, 256], f8, "ws1")
        ws2 = load_const(w_s2, [128, 2, 4, 256], f8, "ws2")
        wa1 = load_const(w_a1, [128, 4, 4, 256], f8, "wa1")
        wa2 = load_const(w_a2, [128, 2, 4, 256], f8, "wa2")
        wc1t = load_const(w_c1t, [128, 4, 4, 256], f8, "wc1t")
        wc1b = load_const(w_c1b, [128, 8, 512], f8, "wc1b")
        wc2 = load_const(w_c2, [128, 4, 512], hf, "wc2")
        wg1 = load_const(w_g1, [128, 8, 512], hf, "wg1")
        wg2 = load_const(w_g2, [128, 4, 512], hf, "wg2")
        wo = load_const(w_o, [128, 4, 4], hf, "wo")
        bsb = load_const(biases, [128, 8, 4], f32, "bsb")
        BIDX = {n: i for i, n in enumerate(
            ["b_s1", "b_s2", "b_a1", "b_a2", "b_c1", "b_c2", "b_g1", "b_g2"])}

        def bias_ap(n, nf):
            return bsb[:, BIDX[n], nf:nf + 1]

        bc2row = load_const(bc2_d, [1, D], hf, "bc2row")
        biasT = load_const(biasT_d, [128, 2, 256], hf, "biasT")

        srows = const.tile([2 * BL, 512], f32r, name="srows")

        # ---------------- helpers ----------------
        def act_relu(engine_pick, dst_ap, src_ap, bias):
            if engine_pick == 0:
                nc.scalar.activation(dst_ap, src_ap, AF.Relu, bias=bias)
            else:
                nc.vector.tensor_scalar(dst_ap, src_ap, bias, 0.0,
                                        op0=ALU.add, op1=ALU.max)

        def copy_ps(engine_pick, dst_ap, src_ap):
            if engine_pick == 0:
                nc.scalar.copy(dst_ap, src_ap)
            else:
                nc.vector.tensor_copy(dst_ap, src_ap)

        def mlp_fm(src, nkc, w, bname, dst, name):
            """feature-major MLP layer: dst[:,nf,:] = relu(w.T @ src + bias)."""
            npair = nkc // 2
            for nf in range(4):
                pm = mm512_ps(f"{name}_nf{nf}")
                for i in range(npair):
                    nc.tensor.matmul(pm[:], lhsT=w[:, i, nf, :],
                                     rhs=src[:, 2 * i:2 * i + 2, :],
                                     start=(i == 0), stop=(i == npair - 1),
                                     perf_mode=DRI)
                act_relu(nf % 2, dst[:, nf, :], pm[:], bias_ap(bname, nf))

        def softmax_tail(ET, dstT, name):
            """rowsum over partitions -> approx recip -> outer-bcast -> mult.
            ET [128, 2, 256] hf; dstT [128, 2, 256] f8."""
            prs = rsum_ps(f"rs_{name}")
            for kc in range(2):
                nc.tensor.matmul(prs[:], lhsT=ones_col[:], rhs=ET[:, kc, :],
                                 start=(kc == 0), stop=(kc == 1))
            rec32 = work.tile([1, 256], f32, tag="rec32", bufs=4, name=f"rec32_{name}")
            nc.vector.reciprocal_approx_fast(rec32[:], prs[:])
            rec16 = work.tile([1, 256], hf, tag="rec16", bufs=4, name=f"rec16_{name}")
            nc.vector.tensor_copy(rec16[:], rec32[:])
            prb = attn_ps(f"rb_{name}")
            nc.tensor.matmul(prb[:], lhsT=ones[:], rhs=rec16[:],
                             start=True, stop=True)
            for kc in range(2):
                nc.vector.tensor_tensor(dstT[:, kc, :], ET[:, kc, :], prb[:],
                                        op=ALU.mult)

        # ---------------- pipeline stages ----------------
        state = {}

        def emitA(b):
            st = {}
            # xT -> cmpin kc 0..3 (feature-major cat, both seqs)
            cmpin = work.tile([128, 8, 512], f8, tag="cmpin", bufs=4, name=f"cmpin{b}")
            for s in range(2):
                for tcn in range(2):
                    for dc in range(4):
                        ptr = trT_ps(f"xT{b}_{s}{tcn}{dc}")
                        nc.tensor.matmul(ptr[:], lhsT=xembs[b][s][:, tcn, dc * 128:(dc + 1) * 128],
                                         rhs=ident8[:], start=True, stop=True)
                        dst = cmpin[:, dc, s * 256 + tcn * 128:s * 256 + (tcn + 1) * 128]
                        copy_ps((dc + tcn) % 2, dst, ptr[:])
            # self MLP
            hmid = work.tile([128, 4, 512], f8, tag="mid", bufs=3, name=f"h1_{b}")
            mlp_fm(cmpin, 4, ws1, "b_s1", hmid, f"sm1_{b}")
            qb = work.tile([128, 4, 512], f8, tag="qpq", bufs=3, name=f"q_{b}")
            mlp_fm(hmid, 4, ws2, "b_s2", qb, f"sm2_{b}")
            # self attention scores, k-major; exp per (s, kc)
            ETs = []
            for s in range(2):
                pS = []
                for kc in range(2):
                    p = attn_ps(f"S{b}_{s}{kc}")
                    nc.tensor.matmul(p[:], lhsT=identb[:], rhs=biasT[:, kc, :],
                                     start=True, stop=False, skip_group_check=True)
                    for kp in range(2):
                        nc.tensor.matmul(p[:], lhsT=qb[:, 2 * kp:2 * kp + 2, s * 256 + kc * 128:s * 256 + (kc + 1) * 128],
                                         rhs=qb[:, 2 * kp:2 * kp + 2, s * 256:(s + 1) * 256],
                                         start=False, stop=(kp == 1), perf_mode=DR,
                                         skip_group_check=True)
                    pS.append(p)
                ET = work.tile([128, 2, 256], hf, tag="ET", bufs=4, name=f"ET{b}_{s}")
                for kc in range(2):
                    nc.scalar.activation(ET[:, kc, :], pS[kc][:], AF.Exp,
                                         bias=mcols[b][:, s, kc:kc + 1])
                ETs.append(ET)
            st["attT"] = []
            for s in range(2):
                aT = work.tile([128, 2, 256], f8, tag="attT", bufs=8, name=f"attT{b}_{s}")
                softmax_tail(ETs[s], aT, f"att{b}_{s}")
                st["attT"].append(aT)
            st["cmpin"], st["hmid"], st["qb"] = cmpin, hmid, qb
            state[b] = st

        def emitB(b):
            st = state[b]
            cmpin, hmid, qb = st["cmpin"], st["hmid"], st["qb"]
            # ctx feature-major -> cmpin[:, 4+dc, :]
            for s in range(2):
                for dc in range(4):
                    pm = attn_ps(f"ctxT{b}_{s}{dc}")
                    nc.tensor.matmul(pm[:], lhsT=xembs[b][s][:, :, dc * 128:(dc + 1) * 128],
                                     rhs=st["attT"][s][:, 0:2, :], start=True, stop=True,
                                     perf_mode=DR)
                    copy_ps(dc % 2, cmpin[:, 4 + dc, s * 256:(s + 1) * 256], pm[:])
            # inter MLP (K=1024)
            mlp_fm(cmpin, 8, wa1, "b_a1", hmid, f"im1_{b}")
            mlp_fm(hmid, 4, wa2, "b_a2", qb, f"im2_{b}")  # qb = [pq | hk]
            # inter attention, k-major both directions
            ETz = {}
            for (nm, base, rlo, ms) in (("h2p", 0, 256, 0), ("p2h", 256, 0, 1)):
                pZ = []
                for jc in range(2):
                    p = attn_ps(f"z{nm}{b}_{jc}")
                    for kp in range(2):
                        nc.tensor.matmul(p[:], lhsT=qb[:, 2 * kp:2 * kp + 2, base + jc * 128:base + (jc + 1) * 128],
                                         rhs=qb[:, 2 * kp:2 * kp + 2, rlo:rlo + 256],
                                         start=(kp == 0), stop=(kp == 1), perf_mode=DR,
                                         skip_group_check=True)
                    pZ.append(p)
                ET = work.tile([128, 2, 256], hf, tag="ET", bufs=4, name=f"ETz{b}_{nm}")
                for jc in range(2):
                    nc.scalar.activation(ET[:, jc, :], pZ[jc][:], AF.Exp,
                                         bias=mcols[b][:, ms, jc:jc + 1])
                ETz[nm] = ET
            for nm in ("h2p", "p2h"):
                dT = work.tile([128, 2, 256], f8, tag="attT", bufs=8, name=f"{nm}T{b}")
                softmax_tail(ETz[nm], dT, f"{nm}{b}")
                st[nm] = dT

        def emitC(b):
            st = state[b]
            cmpin = st["cmpin"]
            p2hT, h2pT = st["p2h"], st["h2p"]
            # Y = cat @ Wc1_bot (token-major out)
            Yt = work.tile([128, 4, 512], f8, tag="Y", bufs=3, name=f"Y{b}")
            for s in range(2):
                for tcn in range(2):
                    pm = mm512_ps(f"Y{b}_{s}{tcn}")
                    for kp in range(4):
                        nc.tensor.matmul(pm[:], lhsT=cmpin[:, 2 * kp:2 * kp + 2, s * 256 + tcn * 128:s * 256 + (tcn + 1) * 128],
                                         rhs=wc1b[:, 2 * kp:2 * kp + 2, :],
                                         start=(kp == 0), stop=(kp == 3), perf_mode=DR)
                    copy_ps(tcn, Yt[:, s * 2 + tcn, :], pm[:])
            # compare L1
            cmp1 = work.tile([128, 4, 512], hf, tag="cmp1", bufs=3, name=f"cmp1_{b}")
            for nf in range(4):
                pm = mm512_ps(f"c1_{b}_nf{nf}")
                for kp in range(4):
                    nc.tensor.matmul(pm[:], lhsT=wc1t[:, kp, nf, :],
                                     rhs=cmpin[:, 2 * kp:2 * kp + 2, :],
                                     start=(kp == 0), stop=False, perf_mode=DRI)
                nc.tensor.matmul(pm[:, 0:256], lhsT=Yt[:, 2:4, nf * 128:(nf + 1) * 128],
                                 rhs=p2hT[:, 0:2, :], start=False, stop=False, perf_mode=DR)
                nc.tensor.matmul(pm[:, 256:512], lhsT=Yt[:, 0:2, nf * 128:(nf + 1) * 128],
                                 rhs=h2pT[:, 0:2, :], start=False, stop=True, perf_mode=DR)
                act_relu(nf % 2, cmp1[:, nf, :], pm[:], bias_ap("b_c1", nf))
            # compare L2 (fp16) + masked sum
            for s in range(2):
                cmp2 = work.tile([128, 2, 512], f8, tag="cmp2", bufs=3, name=f"cmp2_{b}_{s}")
                for tcn in range(2):
                    pm = mm512_ps(f"c2_{b}_{s}{tcn}")
                    nc.tensor.matmul(pm[:], lhsT=ones[:], rhs=bc2row[:], start=True, stop=False)
                    for kc in range(4):
                        nc.tensor.matmul(pm[:], lhsT=cmp1[:, kc, s * 256 + tcn * 128:s * 256 + (tcn + 1) * 128],
                                         rhs=wc2[:, kc, :], start=False, stop=(kc == 3))
                    if tcn == 0:
                        nc.scalar.activation(cmp2[:, tcn, :], pm[:], AF.Relu)
                    else:
                        nc.vector.tensor_scalar(cmp2[:, tcn, :], pm[:], 0.0, None,
                                                op0=ALU.max)
                pa = ps.tile([1, 512], f32, space="PSUM", tag="mm512", bufs=3, name=f"sum{b}_{s}")
                for tcn in range(2):
                    nc.tensor.matmul(pa[:], lhsT=maskfs[b][:, s, tcn, :], rhs=cmp2[:, tcn, :],
                                     start=(tcn == 0), stop=(tcn == 1))
                srow = work.tile([1, 512], f32, tag="sumrow", bufs=3, name=f"srow{b}_{s}")
                nc.vector.tensor_copy(srow[:], pa[:])
                nc.sync.dma_start(srows[s * BL + b:s * BL + b + 1, :].bitcast(f32), srow[:])

            if b == 0 and dbg:
                def tap(name, src_ap):
                    if name in dbg:
                        stg = work.tile(list(dbg[name].shape), f32, tag=f"tap_{name}", bufs=1, name=f"tap{name}")
                        nc.vector.tensor_copy(stg[:], src_ap)
                        nc.sync.dma_start(dbg[name].ap(), stg[:])
                tap("dbg_cmpin", cmpin[:])
                tap("dbg_p2hT", p2hT[:])
                tap("dbg_h2pT", h2pT[:])
                tap("dbg_Y", Yt[:])
                tap("dbg_cmp1", cmp1[:])
            # release per-item state
            del state[b]

        # ---------------- pipelined emission ----------------
        for b in range(nitems):
            emitA(b)
            if b >= 1:
                emitB(b - 1)
            if b >= 2:
                emitC(b - 2)
        if nitems >= 1:
            emitB(nitems - 1)
        if nitems >= 2:
            emitC(nitems - 2)
        if nitems >= 1:
            emitC(nitems - 1)

        # ---------------- aggregate MLP (all items at once, fp16) ----------
        if nitems == BL:
            aggT = work.tile([128, 2, 4, BL], hf, tag="aggT", bufs=1, name="aggT")
            for dc in range(4):
                ptr = ps.tile([128, 2 * BL], f32r, space="PSUM", tag="attn", bufs=5,
                              name=f"aggTr{dc}")
                nc.tensor.matmul(ptr[:], lhsT=srows[:, dc * 128:(dc + 1) * 128],
                                 rhs=identr[0:2 * BL, 0:2 * BL], is_transpose=True,
                                 start=True, stop=True)
                for s in range(2):
                    nc.vector.tensor_copy(aggT[:, s, dc, :],
                                          ptr[:, s * BL:(s + 1) * BL].bitcast(f32))
            agg1 = work.tile([128, 4, BL], hf, tag="agg1", bufs=1, name="agg1")
            for nf in range(4):
                pm = attn_ps(f"g1_{nf}")
                for kc in range(8):
                    nc.tensor.matmul(pm[:, 0:BL], lhsT=wg1[:, kc, nf * 128:(nf + 1) * 128],
                                     rhs=aggT[:, kc // 4, kc % 4, :], start=(kc == 0), stop=(kc == 7))
                nc.scalar.activation(agg1[:, nf, :], pm[:, 0:BL], AF.Relu, bias=bias_ap("b_g1", nf))
            agg2 = work.tile([128, 4, BL], hf, tag="agg2", bufs=1, name="agg2")
            for nf in range(4):
                pm = attn_ps(f"g2_{nf}")
                for kc in range(4):
                    nc.tensor.matmul(pm[:, 0:BL], lhsT=wg2[:, kc, nf * 128:(nf + 1) * 128],
                                     rhs=agg1[:, kc, :], start=(kc == 0), stop=(kc == 3))
                nc.scalar.activation(agg2[:, nf, :], pm[:, 0:BL], AF.Relu, bias=bias_ap("b_g2", nf))
            po = attn_ps("po")
            for kc in range(4):
                nc.tensor.matmul(po[0:BL, 0:4], lhsT=agg2[:, kc, :], rhs=wo[:, kc, :],
                                 start=(kc == 0), stop=(kc == 3))
            osb = work.tile([BL, OUT], f32, tag="osb", bufs=1, name="osb")
            nc.vector.tensor_copy(osb[:], po[0:BL, 0:OUT])
            nc.sync.dma_start(out_d.ap(), osb[:])

        ps.release()
        work.release()
        const.release()

    nc.compile()
    return nc


def _get_program(debug_taps=()):
    key = tuple(n for n, _ in debug_taps)
    if key not in _PROG_CACHE:
        _PROG_CACHE[key] = _build_program(debug_taps)
    return _PROG_CACHE[key]


def kernel(prem_input, hypo_input, embed_W, dist_W,
           Ws1, bs1, Ws2, bs2, Wa1, ba1, Wa2, ba2,
           Wc1, bc1, Wc2, bc2, Wg1, bg1, Wg2, bg2, Wo,
           _debug_taps=(), _trace=False, _tmpdir=None):
    import concourse.mybir as mybir
    from concourse.bass_utils import run_bass_kernel_spmd

    nc = _get_program(_debug_taps)

    f32 = np.float32
    np_f8 = mybir.dt.np(mybir.dt.float8e4)

    def as_hf(a):
        return np.ascontiguousarray(np.asarray(a, f32).astype(np.float16))

    def as_f8(a):
        return np.ascontiguousarray(np.asarray(a, f32).astype(np_f8))

    def pack_km(a, dtype):
        """[K, 512] -> [128, K//128, 512]: partition p, chunk c <- row c*128+p."""
        W = np.asarray(a, f32).astype(dtype)
        K = W.shape[0]
        return np.ascontiguousarray(W.reshape(K // 128, 128, W.shape[1]).transpose(1, 0, 2))

    def as_dri(a):
        """fp8 weight [K, 512] -> DoubleRowSwInterleave stationary layout."""
        W = np.asarray(a, f32).astype(np_f8)
        K = W.shape[0]
        t = W.reshape(K // 128, 128, 4, 128)          # [kc, p, nf, m]
        rev = t[:, :, :, ::-1]
        out = np.empty((128, K // 256, 4, 256), np_f8)
        out[:, :, :, 0::2] = rev[0::2].transpose(1, 0, 2, 3)
        out[:, :, :, 1::2] = rev[1::2].transpose(1, 0, 2, 3)
        return np.ascontiguousarray(out)

    def pack_bias(*bs):
        return np.ascontiguousarray(
            np.stack([np.asarray(b, f32).reshape(4, 128).T for b in bs], axis=1))

    # transposed Toeplitz relative-distance bias (+ -30000 diagonal), fp16
    dW = np.asarray(dist_W, f32).reshape(-1)
    strip = np.empty(2 * L - 1, f32)
    strip[:L - 1 - MAX_DIST] = dW[0]
    strip[L - 1 - MAX_DIST:L + MAX_DIST] = dW
    strip[L + MAX_DIST:] = dW[2 * MAX_DIST]
    strip[L - 1] = DIAG_VAL
    p_i = np.arange(128)[:, None]
    q_i = np.arange(256)[None, :]
    biasT = np.empty((128, 2, 256), np.float16)
    for ic in range(2):
        biasT[:, ic, :] = strip[255 + 128 * ic + p_i - q_i].astype(np.float16)

    wo_h = np.zeros((128, 4, 4), np.float16)
    wo_h[:, :, :OUT] = np.asarray(Wo, f32).reshape(4, 128, OUT).transpose(1, 0, 2)

    Wc1f = np.asarray(Wc1, f32)
    common = {
        "emb": as_f8(embed_W),
        "w_s1": as_dri(Ws1), "w_s2": as_dri(Ws2),
        "w_a1": as_dri(Wa1), "w_a2": as_dri(Wa2),
        "w_c1t": as_dri(Wc1f[:2 * E]), "w_c1b": pack_km(Wc1f[2 * E:], np_f8),
        "w_c2": pack_km(Wc2, np.float16),
        "w_g1": pack_km(Wg1, np.float16), "w_g2": pack_km(Wg2, np.float16),
        "w_o": np.ascontiguousarray(wo_h),
        "biases": pack_bias(bs1, bs2, ba1, ba2, bc1, bc2, bg1, bg2),
        "bc2row": as_hf(np.asarray(bc2, f32).reshape(1, D)),
        "biasT": np.ascontiguousarray(biasT),
    }
    prem = np.ascontiguousarray(np.asarray(prem_input).reshape(B, L).astype(np.int32))
    hypo = np.ascontiguousarray(np.asarray(hypo_input).reshape(B, L).astype(np.int32))

    in_maps = []
    for c in range(NCORES):
        sl = slice(c * BL, (c + 1) * BL)
        tokc = np.stack([prem[sl], hypo[sl]], axis=0)  # [2, BL, L]
        in_maps.append({"tok": np.ascontiguousarray(tokc), **common})

    kwargs = {}
    if _trace:
        kwargs.update(trace=True, tmpdir=_tmpdir)
    res = run_bass_kernel_spmd(nc, in_maps, core_ids=list(range(NCORES)), **kwargs)
    out = np.concatenate([r["out"] for r in res.results], axis=0)
    if _debug_taps or _trace:
        return out, res
    return out
